# revision 33
# baseline (speedup 1.0000x reference)
"""Multi-head attention Trainium2 Bass kernel (v2 — pipelined).

Problem: x[8,1024,768], qkv_w[2304,768], qkv_b[2304], proj_w[768,768],
proj_b[768] -> out[8,1024,768]  (12 heads, head_dim 64, softmax scale 1/8).

Sharding: data-parallel over batch — one batch element per NeuronCore.
Host-side layout prep (part of the sharding strategy): x is passed
transposed per core (xT[c,n]), weights transposed (wT[c,d'], pwT[c,c']).
Two mathematically-exact simplifications:
  - K bias dropped (softmax is invariant to a per-query constant shift).
  - V bias folded into the proj bias: pb = proj_b + proj_w @ v_bias
    (attention rows sum to 1, so the V bias becomes a constant output add).

Per-core pipeline (matmuls bf16, fp32 PSUM; the attention phase is
paced by the ACT-engine exp stream, everything else hides behind it):
  1. Weights cast-DMA'd f32->bf16 via SWDGE (gpsimd); x split across the
     sync HWDGE queue (f32 + DVE cast) and SWDGE so both run in parallel.
  2. Q/K produced transposed qT/kT[d,n]; V natural [n,d] with a ones
     column per head (yields the softmax denominator for free in PV).
  3. Per head-pair: scores sT[j,i] = kT.T@qT into a 3-deep ring of
     [128,1024] PSUM tiles (decouples PE from the exp drain); exp on ACT;
     the 4 PV passes (half x ic) of pair t are deferred into pair t+1's
     jt loop as PE fillers, interleaved with Q/K for pair t+2.  All
     staged tensors are split into per-stage tiles because Tile's RAW
     tracking is tile-coarse.
  4. Denominators: DVE reciprocal costs ~6.4ns per free-dim element, so
     the den rows are resprayed to [128, 16] by DMA, recip'd, then
     DMA-broadcast (via DRAM) across each 64-partition half.
  5. proj reads normalized attnU with a split accumulation (ct 0-4
     partials, ct 5 finisher) so it can start while the last pair's
     normalize chain drains; output over 4 DMAs on sync/scalar queues.
"""

import sys

if "/opt/trn_rl_repo" not in sys.path:
    sys.path.insert(0, "/opt/trn_rl_repo")

from contextlib import ExitStack

import numpy as np

import concourse.bass as bass
import concourse.tile as tile
from concourse import mybir
from concourse.bass_utils import run_bass_kernel_spmd

F32 = mybir.dt.float32
BF16 = mybir.dt.bfloat16
AF = mybir.ActivationFunctionType


def _split_dma_waits(nc: bass.Bass):
    """TRN2 instruction encodings hold at most 1 sync-wait (EventSemaphore: 2),
    but Tile can attach several. Hoist all but one wait onto single-wait NoOps
    inserted just before on the same engine."""
    for f in nc.m.functions:
        for blk in f.blocks:
            insts = blk.instructions
            i = 0
            while i < len(insts):
                inst = insts[i]
                limit = 2 if isinstance(inst, mybir.InstEventSemaphore) else 1
                if (inst.sync_info is not None
                        and len(inst.sync_info.on_wait) > limit):
                    waits = list(inst.sync_info.on_wait)
                    inst.sync_info = mybir.SyncInfo(
                        on_wait=waits[-limit:],
                        on_update=list(inst.sync_info.on_update))
                    for w in waits[:-limit]:
                        nop = mybir.InstNoOp(
                            name=nc.get_next_instruction_name(),
                            ins=[], outs=[])
                        nop.engine = inst.engine
                        nop.sync_info = mybir.SyncInfo(
                            on_wait=[w], on_update=[])
                        insts.insert(i, nop)
                        i += 1
                i += 1


B, N, C = 8, 1024, 768
H, HD = 12, 64
D3 = 3 * C
SCALE = HD ** -0.5
NT = N // 128   # 8 token tiles
CT = C // 128   # 6 channel tiles
NPAIR = H // 2  # 6 head pairs


def build_kernel(nc: bass.Bass):
    xT = nc.dram_tensor("xT", [C, N], F32, kind="ExternalInput").ap()
    wT = nc.dram_tensor("wT", [C, D3], F32, kind="ExternalInput").ap()
    pwT = nc.dram_tensor("pwT", [C, C], F32, kind="ExternalInput").ap()
    qb = nc.dram_tensor("qb", [128, CT], F32, kind="ExternalInput").ap()
    pb = nc.dram_tensor("pb", [C], F32, kind="ExternalInput").ap()
    out = nc.dram_tensor("out", [N, C], F32, kind="ExternalOutput").ap()

    def bcast_ap(src: bass.AP, parts: int) -> bass.AP:
        return bass.AP(tensor=src.tensor, offset=src.offset,
                       ap=[[0, parts], *src.ap])

    with tile.TileContext(nc) as tc, ExitStack() as ctx:
        consts = ctx.enter_context(tc.tile_pool(name="consts", bufs=1))
        expp0 = ctx.enter_context(tc.tile_pool(name="expp0", bufs=10))
        expp1 = ctx.enter_context(tc.tile_pool(name="expp1", bufs=14))
        dstgp = ctx.enter_context(tc.tile_pool(name="dstg", bufs=2))
        rbcp = ctx.enter_context(tc.tile_pool(name="rbc", bufs=1))
        osbp = ctx.enter_context(tc.tile_pool(name="osb", bufs=2))
        ps_s = ctx.enter_context(tc.tile_pool(name="ps_s", bufs=3, space="PSUM"))
        ps_w = ctx.enter_context(tc.tile_pool(name="ps_w", bufs=2, space="PSUM"))
        dram = ctx.enter_context(tc.tile_pool(name="dram", bufs=2, space="DRAM"))

        # ---- persistent bf16 operands --------------------------------
        # Tile RAW tracking is tile-coarse (a read waits on ALL prior
        # writes to the tile), so anything written in stages is split
        # into per-stage tiles to avoid false serialization.
        xTs = consts.tile([128, CT, N], BF16)       # x.T  [c, n]
        wq0 = consts.tile([128, CT, 128], BF16)     # q cols, d-tile 0
        wk0 = consts.tile([128, CT, 128], BF16)     # k cols, d-tile 0
        wqr = consts.tile([128, CT, 5 * 128], BF16)  # q cols, d-tiles 1-5
        wkr = consts.tile([128, CT, 5 * 128], BF16)  # k cols, d-tiles 1-5
        wv = consts.tile([128, CT, C], BF16)        # qkv_w.T v cols
        pwTs = consts.tile([128, CT, C], BF16)      # proj_w.T
        qTt = [consts.tile([128, N], BF16, name=f"qT{i}") for i in range(CT)]
        kTt = [consts.tile([128, N], BF16, name=f"kT{i}") for i in range(CT)]
        v_sb = consts.tile([128, NT, H, HD + 1], BF16)  # v + ones col
        attnU = [consts.tile([128, N], BF16, name=f"aU{i}") for i in range(CT)]
        qbs = consts.tile([128, CT], F32)           # q bias [p, t]
        pbb = consts.tile([128, C], F32)            # proj(+v) bias bcast
        wrm = consts.tile([1, 8], F32)
        wrm2 = consts.tile([1, 8], BF16, name="wrm2")

        # ---- warmup exp: pull the ACT table load to t=0 ---------------
        nc.vector.memset(wrm, 0.0)
        nc.scalar.activation(out=wrm2, in_=wrm, func=AF.Exp, scale=SCALE)
        nc.vector.memset(v_sb[:, :, :, HD:HD + 1], 1.0)

        # ---- input DMAs ----------------------------------------------
        # x + weights via SWDGE cast-DMA (f32 -> bf16), ordered so pair 0
        # can start asap: x per-ct, then just the d-tile-0 q/k columns,
        # then wv (pair-0 V fillers), then the remaining q/k, then proj_w.
        # x: low half on sync HWDGE (f32 + DVE cast), high half on SWDGE —
        # the two queues run in parallel so x is resident ~5us sooner.
        nc.gpsimd.dma_start(
            out=xTs[:, 3:6, :],
            in_=xT[384:768, :].rearrange("(ct p) n -> p ct n", p=128))
        xs = consts.tile([128, 3, N], F32, name="xstage")
        nc.sync.dma_start(
            out=xs, in_=xT[0:384, :].rearrange("(ct p) n -> p ct n", p=128))
        nc.vector.tensor_copy(out=xTs[:, 0:3, :], in_=xs)
        # d-tile-0 q/k columns on the scalar HWDGE queue (f32 + DVE cast);
        # the slow-descriptor bias loads go LAST on that queue.
        wq0s = consts.tile([128, CT, 128], F32, name="wq0s")
        wk0s = consts.tile([128, CT, 128], F32, name="wk0s")
        nc.scalar.dma_start(
            out=wq0s, in_=wT[:, 0:128].rearrange("(ct p) d -> p ct d", p=128))
        nc.vector.tensor_copy(out=wq0, in_=wq0s)
        nc.scalar.dma_start(
            out=wk0s,
            in_=wT[:, C:C + 128].rearrange("(ct p) d -> p ct d", p=128))
        nc.vector.tensor_copy(out=wk0, in_=wk0s)
        nc.scalar.dma_start(out=qbs, in_=qb)
        nc.scalar.dma_start(out=pbb, in_=bcast_ap(pb, 128))
        nc.gpsimd.dma_start(
            out=wv, in_=wT[:, 2 * C:D3].rearrange("(ct p) d -> p ct d", p=128))
        nc.gpsimd.dma_start(
            out=wqr,
            in_=wT[:, 128:C].rearrange("(ct p) d -> p ct d", p=128))
        nc.gpsimd.dma_start(
            out=wkr,
            in_=wT[:, C + 128:2 * C].rearrange("(ct p) d -> p ct d", p=128))
        nc.gpsimd.dma_start(
            out=pwTs, in_=pwT.rearrange("(ct p) d -> p ct d", p=128))

        # ---- emit helpers --------------------------------------------
        def emit_q(t, ic):
            ps = ps_w.tile([128, 512], F32, tag="w", name="psq")
            w = wq0 if t == 0 else wqr
            toff = 0 if t == 0 else (t - 1) * 128
            for ct in range(CT):
                nc.tensor.matmul(
                    ps, lhsT=w[:, ct, toff:toff + 128],
                    rhs=xTs[:, ct, ic * 512:(ic + 1) * 512],
                    start=(ct == 0), stop=(ct == CT - 1))
            nc.vector.tensor_scalar_add(
                out=qTt[t][:, ic * 512:(ic + 1) * 512], in0=ps,
                scalar1=qbs[:, t:t + 1])

        def emit_k(t, ic):
            ps = ps_w.tile([128, 512], F32, tag="w", name="psk")
            w = wk0 if t == 0 else wkr
            toff = 0 if t == 0 else (t - 1) * 128
            for ct in range(CT):
                nc.tensor.matmul(
                    ps, lhsT=w[:, ct, toff:toff + 128],
                    rhs=xTs[:, ct, ic * 512:(ic + 1) * 512],
                    start=(ct == 0), stop=(ct == CT - 1))
            nc.vector.tensor_copy(
                out=kTt[t][:, ic * 512:(ic + 1) * 512], in_=ps)

        def emit_v_chunk(t, half):
            lo, hi, h0, hn = ((0, 512, 0, 8), (512, 768, 8, 4))[half]
            ps = ps_w.tile([128, 512], F32, tag="w", name="psv")
            for ct in range(CT):
                nc.tensor.matmul(
                    ps[:, 0:hi - lo],
                    lhsT=xTs[:, ct, t * 128:(t + 1) * 128],
                    rhs=wv[:, ct, lo:hi],
                    start=(ct == 0), stop=(ct == CT - 1))
            nc.vector.tensor_copy(
                out=v_sb[:, t, h0:h0 + hn, 0:HD],
                in_=ps[:, 0:hi - lo].rearrange("p (h d) -> p h d", h=hn))

        def emit_scores(t, jt):
            # per-half scores tiles (double-buffered in PSUM) so the exp on
            # ACT overlaps the next scores matmuls instead of serializing.
            s_pair = [ps_s.tile([128, 1024], F32, tag="s", name="s")
                      for _ in range(2)]
            for ic in range(2):
                for h in range(2):
                    nc.tensor.matmul(
                        s_pair[h][:, ic * 512:(ic + 1) * 512],
                        lhsT=kTt[t][64 * h:64 * h + 64, jt * 128:(jt + 1) * 128],
                        rhs=qTt[t][64 * h:64 * h + 64, ic * 512:(ic + 1) * 512],
                        start=True, stop=True)
            e_pair = []
            for h in range(2):
                # separate rings per half: h0 tiles are freed (by the
                # deferred PV passes) two jt-steps earlier than h1 tiles,
                # and a shared FIFO ring would serialize on the late ones.
                e = (expp0 if h == 0 else expp1).tile(
                    [128, 1024], BF16, tag=f"e{h}", name="e")
                nc.scalar.activation(out=e, in_=s_pair[h], func=AF.Exp,
                                     scale=SCALE)
                e_pair.append(e)
            return e_pair

        def emit_pv_pass(t, h, ic, e_list, dstg_t):
            ow = ps_w.tile([128, 512], F32, tag="w", name="o")
            o = ow[0:65, :]
            for jt in range(NT):
                nc.tensor.matmul(
                    o, lhsT=v_sb[:, jt, 2 * t + h, :],
                    rhs=e_list[jt][h][:, ic * 512:(ic + 1) * 512],
                    start=(jt == 0), stop=(jt == NT - 1))
            # den copy first: it gates the (serial-DMA) normalize chain,
            # while the attnU copy is only needed by the later proj.
            nc.vector.tensor_copy(
                out=dstg_t[64:65, h, ic * 512:(ic + 1) * 512],
                in_=o[64:65, :])
            nc.vector.tensor_copy(
                out=attnU[t][64 * h:64 * h + 64, ic * 512:(ic + 1) * 512],
                in_=o[0:64, :])

        def emit_normalize_half(t, h, dstg_t, eng):
            # One head's den row lives at partition 64 of dstg.  DVE
            # reciprocal costs ~6.4ns per FREE-dim element, so scatter the
            # 1024 denominators to [128, 8] (SBUF->SBUF DMA respray) for
            # the reciprocal, then DMA-broadcast the RECIPROCALS across
            # this head's 64-partition half.
            rsm = rbcp.tile([128, 8], F32, tag=f"rs{h}", name="rsm")
            eng.dma_start(out=rsm, in_=dstg_t[64:65, h, :])
            nc.vector.reciprocal(out=rsm, in_=rsm)
            dsc = dram.tile([N], F32, tag=f"dsc{h}", name="dsc")
            eng.dma_start(out=dsc.rearrange("(p f) -> p f", p=128), in_=rsm)
            rbc = rbcp.tile([128, N], F32, tag=f"r{h}", name="rbc")
            eng.dma_start(out=rbc[64 * h:64 * h + 64, :], in_=bcast_ap(dsc, 64))
            nc.vector.tensor_mul(
                out=attnU[t][64 * h:64 * h + 64, :],
                in0=attnU[t][64 * h:64 * h + 64, :],
                in1=rbc[64 * h:64 * h + 64, :])

        # ---- phase A: first Q/K ---------------------------------------
        emit_q(0, 0)
        emit_q(0, 1)
        emit_k(0, 0)  # K ic1 (keys 512:1024) is deferred to pair-0 jt0

        # ---- attention: pipelined pairs ------------------------------
        # Filler schedule per jt (pairs >= 1), chosen to spread PE work
        # evenly so HAM never re-throttles:
        #   jt0,jt1: prev pair's h0 PV passes     jt2,jt3: Q(t+1)
        #   jt4,jt5: prev pair's h1 PV passes     jt6,jt7: K(t+1)
        #   jt2: normalize prev h0   jt6: normalize prev h1
        e_hist: dict[int, list] = {}
        dstg_hist: dict[int, object] = {}
        for t in range(NPAIR):
            dstg_hist[t] = dstgp.tile([65, 2, N], F32, tag="d", name="dstg")
            e_list = []
            for jt in range(NT):
                if t == 0:
                    # scores first: the V fillers gate on the (late) wv DMA
                    # and must not block them in PE program order.
                    e_list.append(emit_scores(t, jt))
                    if jt == 0:
                        emit_k(0, 1)
                    if jt >= 2:
                        for vq in range(3):
                            vi = (jt - 2) * 3 + vq
                            if vi < 16:
                                emit_v_chunk(vi // 2, vi % 2)
                    if jt >= 4:
                        (emit_q if jt < 6 else emit_k)(1, jt % 2)
                else:
                    # fillers first: PE chews these while ACT drains the
                    # previous jt's scores tiles.
                    if jt in (0, 1, 4, 5):
                        h, ic = divmod(jt, 4)
                        emit_pv_pass(t - 1, h, ic, e_hist[t - 1],
                                     dstg_hist[t - 1])
                    elif t + 1 < NPAIR:
                        (emit_q if jt < 4 else emit_k)(t + 1, jt % 2)
                    if jt == 2:
                        emit_normalize_half(t - 1, 0, dstg_hist[t - 1],
                                            nc.sync)
                    elif jt == 6:
                        emit_normalize_half(t - 1, 1, dstg_hist[t - 1],
                                            nc.sync)
                    e_list.append(emit_scores(t, jt))
            e_hist[t] = e_list
            if t - 2 >= 0:
                del e_hist[t - 2]  # release python refs (slots recycle anyway)

        # ---- tail: last pair's PV + normalize ------------------------
        tl = NPAIR - 1
        for h in (1, 0):
            for ic in range(2):
                emit_pv_pass(tl, h, ic, e_hist[tl], dstg_hist[tl])
        # single combined chain for the last pair: one scatter+recip+store,
        # two broadcasts on parallel queues, one full-width mul — the last
        # mul gates proj's ct5 matmuls, so chain length is the tail.
        dstg_t = dstg_hist[tl]
        rsm2 = rbcp.tile([128, 16], F32, tag="rs2", name="rsm2")
        nc.sync.dma_start(
            out=rsm2, in_=dstg_t[64:65, :, :].rearrange("p h n -> p (h n)"))
        nc.vector.reciprocal(out=rsm2, in_=rsm2)
        dsc5 = dram.tile([2, N], F32, tag="dsc5", name="dsc5")
        nc.sync.dma_start(
            out=dsc5.rearrange("h n -> (h n)").rearrange("(p f) -> p f",
                                                         p=128),
            in_=rsm2)
        rbc5 = rbcp.tile([128, N], F32, tag="r5", name="rbc5")
        nc.sync.dma_start(out=rbc5[0:64, :], in_=bcast_ap(dsc5[0, :], 64))
        nc.scalar.dma_start(out=rbc5[64:128, :], in_=bcast_ap(dsc5[1, :], 64))
        nc.vector.tensor_mul(
            out=attnU[tl][:, :], in0=attnU[tl][:, :], in1=rbc5)

        # ---- output projection ---------------------------------------
        # ct order puts the straggler pair (ct=5) last, so each chunk's
        # first 5 matmuls can run while its normalize chain drains.
        proj_ctr = [0]

        def emit_proj_partial(nt, lo, hi):
            # ct 0..4 partial accumulation: depends only on pairs 0-4,
            # which normalized long ago — runs during pair-5's chain.
            proj_ctr[0] += 1
            if proj_ctr[0] % 5 < 3:
                psw = ps_s.tile([128, 1024], F32, tag="s", name="pso")
            else:
                psw = ps_w.tile([128, 512], F32, tag="w", name="psow")
            pso = psw[:, 0:512]
            for ct in range(CT - 1):
                nc.tensor.matmul(
                    pso[:, 0:hi - lo],
                    lhsT=attnU[ct][:, nt * 128:(nt + 1) * 128],
                    rhs=pwTs[:, ct, lo:hi],
                    start=(ct == 0), stop=False)
            return psw

        def emit_proj_finish(psw, osb, i2, nt, lo, hi):
            pso = psw[:, 0:512]
            nc.tensor.matmul(
                pso[:, 0:hi - lo],
                lhsT=attnU[CT - 1][:, nt * 128:(nt + 1) * 128],
                rhs=pwTs[:, CT - 1, lo:hi],
                start=False, stop=True, skip_group_check=True)
            nc.vector.tensor_add(
                out=osb[:, i2, lo:hi], in0=pso[:, 0:hi - lo],
                in1=pbb[:, lo:hi])

        def emit_out_dma(g, osb):
            eng = nc.sync if g % 2 == 0 else nc.scalar
            eng.dma_start(
                out=out[g * 256:(g + 1) * 256, :].rearrange(
                    "(t p) c -> p t c", p=128),
                in_=osb)

        units = [(g, i2, lohi) for g in range(4) for i2 in range(2)
                 for lohi in ((0, 512), (512, 768))]
        pending = []
        osb_of = {}
        done_of = {g: 0 for g in range(4)}

        def finish_one():
            psw, g, osb, i2, nt, lo, hi = pending.pop(0)
            emit_proj_finish(psw, osb, i2, nt, lo, hi)
            done_of[g] += 1
            if done_of[g] == 4:
                emit_out_dma(g, osb)

        for (g, i2, (lo, hi)) in units:
            if g not in osb_of:
                osb_of[g] = osbp.tile([128, 2, C], F32, tag="osb", name="osb")
            if len(pending) >= 5:
                finish_one()
            nt = g * 2 + i2
            pending.append(
                (emit_proj_partial(nt, lo, hi), g, osb_of[g], i2, nt, lo, hi))
        while pending:
            finish_one()

    _split_dma_waits(nc)
    return nc


_NC_CACHE = None


def _get_nc():
    global _NC_CACHE
    if _NC_CACHE is None:
        _NC_CACHE = build_kernel(
            bass.Bass("TRN2", target_bir_lowering=False, debug=False))
    return _NC_CACHE


def make_in_maps(inputs: dict) -> list[dict]:
    """Host-side shard/layout prep: transpose per chosen layout, fold biases."""
    x = np.asarray(inputs["x"], dtype=np.float32)
    qkv_w = np.asarray(inputs["qkv_w"], dtype=np.float32)
    qkv_b = np.asarray(inputs["qkv_b"], dtype=np.float32)
    proj_w = np.asarray(inputs["proj_w"], dtype=np.float32)
    proj_b = np.asarray(inputs["proj_b"], dtype=np.float32)
    shared = {
        "wT": np.ascontiguousarray(qkv_w.T),
        "pwT": np.ascontiguousarray(proj_w.T),
        "qb": np.ascontiguousarray(qkv_b[0:C].reshape(CT, 128).T),
        # V bias folded through proj (attention rows sum to 1)
        "pb": np.ascontiguousarray(proj_b + proj_w @ qkv_b[2 * C:D3]),
    }
    return [{"xT": np.ascontiguousarray(x[b].T), **shared} for b in range(B)]


def kernel(**inputs: np.ndarray) -> np.ndarray:
    nc = _get_nc()
    in_maps = make_in_maps(inputs)
    res = run_bass_kernel_spmd(nc, in_maps, core_ids=list(range(B)))
    return np.stack([r["out"] for r in res.results]).astype(np.float32)


if __name__ == "__main__":
    from reference import setup_inputs, reference

    inputs = {k: np.asarray(v) for k, v in setup_inputs().items()}
    got = kernel(**inputs)
    exp = np.asarray(reference(**inputs))
    err = np.abs(got - exp)
    print("abs err max:", err.max(), "ref absmax:", np.abs(exp).max())
    print("rel(absmax):", err.max() / np.abs(exp).max())


# revision 35
# speedup vs baseline: 1.0452x; 1.0452x over previous
"""Multi-head attention Trainium2 Bass kernel (v2 — pipelined).

Problem: x[8,1024,768], qkv_w[2304,768], qkv_b[2304], proj_w[768,768],
proj_b[768] -> out[8,1024,768]  (12 heads, head_dim 64, softmax scale 1/8).

Sharding: data-parallel over batch — one batch element per NeuronCore.
Host-side layout prep (part of the sharding strategy): x is passed
transposed per core (xT[c,n]), weights transposed (wT[c,d'], pwT[c,c']).
Two mathematically-exact simplifications:
  - K bias dropped (softmax is invariant to a per-query constant shift).
  - V bias folded into the proj bias: pb = proj_b + proj_w @ v_bias
    (attention rows sum to 1, so the V bias becomes a constant output add).

Per-core pipeline (matmuls bf16, fp32 PSUM; the attention phase is
paced by the ACT-engine exp stream, everything else hides behind it):
  1. Weights cast-DMA'd f32->bf16 via SWDGE (gpsimd); x split across the
     sync HWDGE queue (f32 + DVE cast) and SWDGE so both run in parallel.
  2. Q/K produced transposed qT/kT[d,n]; V natural [n,d] with a ones
     column per head (yields the softmax denominator for free in PV).
  3. Per head-pair: scores sT[j,i] = kT.T@qT into a 3-deep ring of
     [128,1024] PSUM tiles (decouples PE from the exp drain); exp on ACT;
     the 4 PV passes (half x ic) of pair t are deferred into pair t+1's
     jt loop as PE fillers, interleaved with Q/K for pair t+2.  All
     staged tensors are split into per-stage tiles because Tile's RAW
     tracking is tile-coarse.
  4. Denominators: DVE reciprocal costs ~6.4ns per free-dim element, so
     the den rows are resprayed to [128, 16] by DMA, recip'd, then
     DMA-broadcast (via DRAM) across each 64-partition half.
  5. proj reads normalized attnU with a split accumulation (ct 0-4
     partials, ct 5 finisher) so it can start while the last pair's
     normalize chain drains; output over 4 DMAs on sync/scalar queues.
"""

import sys

if "/opt/trn_rl_repo" not in sys.path:
    sys.path.insert(0, "/opt/trn_rl_repo")

from contextlib import ExitStack

import numpy as np

import concourse.bass as bass
import concourse.tile as tile
from concourse import mybir
from concourse.bass_utils import run_bass_kernel_spmd

F32 = mybir.dt.float32
BF16 = mybir.dt.bfloat16
AF = mybir.ActivationFunctionType


def _split_dma_waits(nc: bass.Bass):
    """TRN2 instruction encodings hold at most 1 sync-wait (EventSemaphore: 2),
    but Tile can attach several. Hoist all but one wait onto single-wait NoOps
    inserted just before on the same engine."""
    for f in nc.m.functions:
        for blk in f.blocks:
            insts = blk.instructions
            i = 0
            while i < len(insts):
                inst = insts[i]
                limit = 2 if isinstance(inst, mybir.InstEventSemaphore) else 1
                if (inst.sync_info is not None
                        and len(inst.sync_info.on_wait) > limit):
                    waits = list(inst.sync_info.on_wait)
                    inst.sync_info = mybir.SyncInfo(
                        on_wait=waits[-limit:],
                        on_update=list(inst.sync_info.on_update))
                    for w in waits[:-limit]:
                        nop = mybir.InstNoOp(
                            name=nc.get_next_instruction_name(),
                            ins=[], outs=[])
                        nop.engine = inst.engine
                        nop.sync_info = mybir.SyncInfo(
                            on_wait=[w], on_update=[])
                        insts.insert(i, nop)
                        i += 1
                i += 1


B, N, C = 8, 1024, 768
H, HD = 12, 64
D3 = 3 * C
SCALE = HD ** -0.5
NT = N // 128   # 8 token tiles
CT = C // 128   # 6 channel tiles
NPAIR = H // 2  # 6 head pairs


def build_kernel(nc: bass.Bass):
    xT = nc.dram_tensor("xT", [C, N], F32, kind="ExternalInput").ap()
    wT = nc.dram_tensor("wT", [C, D3], F32, kind="ExternalInput").ap()
    pwT = nc.dram_tensor("pwT", [C, C], F32, kind="ExternalInput").ap()
    qb = nc.dram_tensor("qb", [128, CT], F32, kind="ExternalInput").ap()
    pb = nc.dram_tensor("pb", [C], F32, kind="ExternalInput").ap()
    out = nc.dram_tensor("out", [N, C], F32, kind="ExternalOutput").ap()

    def bcast_ap(src: bass.AP, parts: int) -> bass.AP:
        return bass.AP(tensor=src.tensor, offset=src.offset,
                       ap=[[0, parts], *src.ap])

    with tile.TileContext(nc) as tc, ExitStack() as ctx:
        consts = ctx.enter_context(tc.tile_pool(name="consts", bufs=1))
        expp0 = ctx.enter_context(tc.tile_pool(name="expp0", bufs=10))
        expp1 = ctx.enter_context(tc.tile_pool(name="expp1", bufs=14))
        dstgp = ctx.enter_context(tc.tile_pool(name="dstg", bufs=2))
        rbcp = ctx.enter_context(tc.tile_pool(name="rbc", bufs=1))
        osbp = ctx.enter_context(tc.tile_pool(name="osb", bufs=2))
        ps_s = ctx.enter_context(tc.tile_pool(name="ps_s", bufs=3, space="PSUM"))
        ps_w = ctx.enter_context(tc.tile_pool(name="ps_w", bufs=2, space="PSUM"))
        dram = ctx.enter_context(tc.tile_pool(name="dram", bufs=2, space="DRAM"))

        # ---- persistent bf16 operands --------------------------------
        # Tile RAW tracking is tile-coarse (a read waits on ALL prior
        # writes to the tile), so anything written in stages is split
        # into per-stage tiles to avoid false serialization.
        xTs = consts.tile([128, CT, N], BF16)       # x.T  [c, n]
        wq0 = consts.tile([128, CT, 128], BF16)     # q cols, d-tile 0
        wk0 = consts.tile([128, CT, 128], BF16)     # k cols, d-tile 0
        wqr = consts.tile([128, CT, 5 * 128], BF16)  # q cols, d-tiles 1-5
        wkr = consts.tile([128, CT, 5 * 128], BF16)  # k cols, d-tiles 1-5
        wv = consts.tile([128, CT, C], BF16)        # qkv_w.T v cols
        pwTs = consts.tile([128, CT, C], BF16)      # proj_w.T
        qTt = [consts.tile([128, N], BF16, name=f"qT{i}") for i in range(CT)]
        kTt = [consts.tile([128, N], BF16, name=f"kT{i}") for i in range(CT)]
        v_sb = consts.tile([128, NT, H, HD + 1], BF16)  # v + ones col
        attnU = [consts.tile([128, N], BF16, name=f"aU{i}") for i in range(CT)]
        qbs = consts.tile([128, CT], F32)           # q bias [p, t]
        pbb = consts.tile([128, C], F32)            # proj(+v) bias bcast
        wrm = consts.tile([1, 8], F32)
        wrm2 = consts.tile([1, 8], BF16, name="wrm2")

        # ---- warmup exp: pull the ACT table load to t=0 ---------------
        nc.vector.memset(wrm, 0.0)
        nc.scalar.activation(out=wrm2, in_=wrm, func=AF.Exp, scale=SCALE)
        nc.vector.memset(v_sb[:, :, :, HD:HD + 1], 1.0)

        # ---- input DMAs ----------------------------------------------
        # x + weights via SWDGE cast-DMA (f32 -> bf16), ordered so pair 0
        # can start asap: x per-ct, then just the d-tile-0 q/k columns,
        # then wv (pair-0 V fillers), then the remaining q/k, then proj_w.
        # x: low half on sync HWDGE (f32 + DVE cast), high half on SWDGE —
        # the two queues run in parallel so x is resident ~5us sooner.
        nc.gpsimd.dma_start(
            out=xTs[:, 2:6, :],
            in_=xT[256:768, :].rearrange("(ct p) n -> p ct n", p=128))
        xs = consts.tile([128, 2, N], F32, name="xstage")
        nc.sync.dma_start(
            out=xs, in_=xT[0:256, :].rearrange("(ct p) n -> p ct n", p=128))
        # d-tile-0 q/k columns on the scalar HWDGE queue (f32 + DVE cast);
        # the slow-descriptor bias loads go LAST on that queue.  Their DVE
        # casts are emitted BEFORE the (slower) x cast so wq0 is ready early.
        wq0s = consts.tile([128, CT, 128], F32, name="wq0s")
        wk0s = consts.tile([128, CT, 128], F32, name="wk0s")
        nc.scalar.dma_start(
            out=wq0s, in_=wT[:, 0:128].rearrange("(ct p) d -> p ct d", p=128))
        nc.vector.tensor_copy(out=wq0, in_=wq0s)
        nc.scalar.dma_start(
            out=wk0s,
            in_=wT[:, C:C + 128].rearrange("(ct p) d -> p ct d", p=128))
        nc.vector.tensor_copy(out=wk0, in_=wk0s)
        nc.vector.tensor_copy(out=xTs[:, 0:2, :], in_=xs)
        # PE warm-up: dependency-light matmuls on wq0 bridge the x-DMA wait
        # so the HAM clock gate is already open (2.4GHz) when the real
        # Q0/K0 chain issues.  Results are garbage and never read.
        wrmps = ps_w.tile([128, 512], F32, tag="w", name="wrmps")
        for _ in range(24):
            nc.tensor.matmul(
                wrmps, lhsT=wq0[:, 0, :],
                rhs=wq0[:, 0:4, :].rearrange("p a b -> p (a b)"),
                start=True, stop=True)
        nc.scalar.dma_start(out=qbs, in_=qb)
        nc.scalar.dma_start(out=pbb, in_=bcast_ap(pb, 128))
        nc.gpsimd.dma_start(
            out=wv, in_=wT[:, 2 * C:D3].rearrange("(ct p) d -> p ct d", p=128))
        nc.gpsimd.dma_start(
            out=wqr,
            in_=wT[:, 128:C].rearrange("(ct p) d -> p ct d", p=128))
        nc.gpsimd.dma_start(
            out=wkr,
            in_=wT[:, C + 128:2 * C].rearrange("(ct p) d -> p ct d", p=128))
        nc.gpsimd.dma_start(
            out=pwTs, in_=pwT.rearrange("(ct p) d -> p ct d", p=128))

        # ---- emit helpers --------------------------------------------
        def emit_q(t, ic):
            ps = ps_w.tile([128, 512], F32, tag="w", name="psq")
            w = wq0 if t == 0 else wqr
            toff = 0 if t == 0 else (t - 1) * 128
            for ct in range(CT):
                nc.tensor.matmul(
                    ps, lhsT=w[:, ct, toff:toff + 128],
                    rhs=xTs[:, ct, ic * 512:(ic + 1) * 512],
                    start=(ct == 0), stop=(ct == CT - 1))
            nc.vector.tensor_scalar_add(
                out=qTt[t][:, ic * 512:(ic + 1) * 512], in0=ps,
                scalar1=qbs[:, t:t + 1])

        def emit_k(t, ic):
            ps = ps_w.tile([128, 512], F32, tag="w", name="psk")
            w = wk0 if t == 0 else wkr
            toff = 0 if t == 0 else (t - 1) * 128
            for ct in range(CT):
                nc.tensor.matmul(
                    ps, lhsT=w[:, ct, toff:toff + 128],
                    rhs=xTs[:, ct, ic * 512:(ic + 1) * 512],
                    start=(ct == 0), stop=(ct == CT - 1))
            nc.vector.tensor_copy(
                out=kTt[t][:, ic * 512:(ic + 1) * 512], in_=ps)

        def emit_v_chunk(t, half):
            lo, hi, h0, hn = ((0, 512, 0, 8), (512, 768, 8, 4))[half]
            ps = ps_w.tile([128, 512], F32, tag="w", name="psv")
            for ct in range(CT):
                nc.tensor.matmul(
                    ps[:, 0:hi - lo],
                    lhsT=xTs[:, ct, t * 128:(t + 1) * 128],
                    rhs=wv[:, ct, lo:hi],
                    start=(ct == 0), stop=(ct == CT - 1))
            nc.vector.tensor_copy(
                out=v_sb[:, t, h0:h0 + hn, 0:HD],
                in_=ps[:, 0:hi - lo].rearrange("p (h d) -> p h d", h=hn))

        def emit_scores(t, jt):
            # per-half scores tiles (double-buffered in PSUM) so the exp on
            # ACT overlaps the next scores matmuls instead of serializing.
            s_pair = [ps_s.tile([128, 1024], F32, tag="s", name="s")
                      for _ in range(2)]
            for ic in range(2):
                for h in range(2):
                    nc.tensor.matmul(
                        s_pair[h][:, ic * 512:(ic + 1) * 512],
                        lhsT=kTt[t][64 * h:64 * h + 64, jt * 128:(jt + 1) * 128],
                        rhs=qTt[t][64 * h:64 * h + 64, ic * 512:(ic + 1) * 512],
                        start=True, stop=True)
            e_pair = []
            for h in range(2):
                # separate rings per half: h0 tiles are freed (by the
                # deferred PV passes) two jt-steps earlier than h1 tiles,
                # and a shared FIFO ring would serialize on the late ones.
                e = (expp0 if h == 0 else expp1).tile(
                    [128, 1024], BF16, tag=f"e{h}", name="e")
                nc.scalar.activation(out=e, in_=s_pair[h], func=AF.Exp,
                                     scale=SCALE)
                e_pair.append(e)
            return e_pair

        def emit_pv_pass(t, h, ic, e_list, dstg_t):
            ow = ps_w.tile([128, 512], F32, tag="w", name="o")
            o = ow[0:65, :]
            for jt in range(NT):
                nc.tensor.matmul(
                    o, lhsT=v_sb[:, jt, 2 * t + h, :],
                    rhs=e_list[jt][h][:, ic * 512:(ic + 1) * 512],
                    start=(jt == 0), stop=(jt == NT - 1))
            # den copy first: it gates the (serial-DMA) normalize chain,
            # while the attnU copy is only needed by the later proj.
            nc.vector.tensor_copy(
                out=dstg_t[64:65, h, ic * 512:(ic + 1) * 512],
                in_=o[64:65, :])
            nc.vector.tensor_copy(
                out=attnU[t][64 * h:64 * h + 64, ic * 512:(ic + 1) * 512],
                in_=o[0:64, :])

        def emit_normalize_half(t, h, dstg_t, eng):
            # One head's den row lives at partition 64 of dstg.  DVE
            # reciprocal costs ~6.4ns per FREE-dim element, so scatter the
            # 1024 denominators to [128, 8] (SBUF->SBUF DMA respray) for
            # the reciprocal, then DMA-broadcast the RECIPROCALS across
            # this head's 64-partition half.
            rsm = rbcp.tile([128, 8], F32, tag=f"rs{h}", name="rsm")
            eng.dma_start(out=rsm, in_=dstg_t[64:65, h, :])
            nc.vector.reciprocal(out=rsm, in_=rsm)
            dsc = dram.tile([N], F32, tag=f"dsc{h}", name="dsc")
            eng.dma_start(out=dsc.rearrange("(p f) -> p f", p=128), in_=rsm)
            rbc = rbcp.tile([128, N], F32, tag=f"r{h}", name="rbc")
            eng.dma_start(out=rbc[64 * h:64 * h + 64, :], in_=bcast_ap(dsc, 64))
            nc.vector.tensor_mul(
                out=attnU[t][64 * h:64 * h + 64, :],
                in0=attnU[t][64 * h:64 * h + 64, :],
                in1=rbc[64 * h:64 * h + 64, :])

        # ---- phase A: first Q/K ---------------------------------------
        emit_q(0, 0)
        emit_q(0, 1)
        emit_k(0, 0)  # K ic1 (keys 512:1024) is deferred to pair-0 jt0

        # ---- attention: pipelined pairs ------------------------------
        # Filler schedule per jt (pairs >= 1), chosen to spread PE work
        # evenly so HAM never re-throttles:
        #   jt0,jt1: prev pair's h0 PV passes     jt2,jt3: Q(t+1)
        #   jt4,jt5: prev pair's h1 PV passes     jt6,jt7: K(t+1)
        #   jt2: normalize prev h0   jt6: normalize prev h1
        e_hist: dict[int, list] = {}
        dstg_hist: dict[int, object] = {}
        for t in range(NPAIR):
            dstg_hist[t] = dstgp.tile([65, 2, N], F32, tag="d", name="dstg")
            e_list = []
            for jt in range(NT):
                if t == 0:
                    # scores first: the V fillers gate on the (late) wv DMA
                    # and must not block them in PE program order.
                    e_list.append(emit_scores(t, jt))
                    if jt == 0:
                        emit_k(0, 1)
                    if jt >= 2:
                        for vq in range(3):
                            vi = (jt - 2) * 3 + vq
                            if vi < 16:
                                emit_v_chunk(vi // 2, vi % 2)
                    if jt >= 4:
                        (emit_q if jt < 6 else emit_k)(1, jt % 2)
                else:
                    # fillers first: PE chews these while ACT drains the
                    # previous jt's scores tiles.
                    if jt in (0, 1, 4, 5):
                        h, ic = divmod(jt, 4)
                        emit_pv_pass(t - 1, h, ic, e_hist[t - 1],
                                     dstg_hist[t - 1])
                    elif t + 1 < NPAIR:
                        (emit_q if jt < 4 else emit_k)(t + 1, jt % 2)
                    if jt == 2:
                        emit_normalize_half(t - 1, 0, dstg_hist[t - 1],
                                            nc.sync)
                    elif jt == 6:
                        emit_normalize_half(t - 1, 1, dstg_hist[t - 1],
                                            nc.sync)
                    e_list.append(emit_scores(t, jt))
            e_hist[t] = e_list
            if t - 2 >= 0:
                del e_hist[t - 2]  # release python refs (slots recycle anyway)

        # ---- tail: last pair's PV + normalize ------------------------
        tl = NPAIR - 1
        for h in (1, 0):
            for ic in range(2):
                emit_pv_pass(tl, h, ic, e_hist[tl], dstg_hist[tl])
        # single combined chain for the last pair: one scatter+recip+store,
        # two broadcasts on parallel queues, one full-width mul — the last
        # mul gates proj's ct5 matmuls, so chain length is the tail.
        dstg_t = dstg_hist[tl]
        rsm2 = rbcp.tile([128, 16], F32, tag="rs2", name="rsm2")
        nc.sync.dma_start(
            out=rsm2, in_=dstg_t[64:65, :, :].rearrange("p h n -> p (h n)"))
        nc.vector.reciprocal(out=rsm2, in_=rsm2)
        dsc5 = dram.tile([2, N], F32, tag="dsc5", name="dsc5")
        nc.sync.dma_start(
            out=dsc5.rearrange("h n -> (h n)").rearrange("(p f) -> p f",
                                                         p=128),
            in_=rsm2)
        rbc5 = rbcp.tile([128, N], F32, tag="r5", name="rbc5")
        nc.sync.dma_start(out=rbc5[0:64, :], in_=bcast_ap(dsc5[0, :], 64))
        nc.scalar.dma_start(out=rbc5[64:128, :], in_=bcast_ap(dsc5[1, :], 64))
        nc.vector.tensor_mul(
            out=attnU[tl][:, :], in0=attnU[tl][:, :], in1=rbc5)

        # ---- output projection ---------------------------------------
        # ct order puts the straggler pair (ct=5) last, so each chunk's
        # first 5 matmuls can run while its normalize chain drains.
        proj_ctr = [0]

        def emit_proj_partial(nt, lo, hi):
            # ct 0..4 partial accumulation: depends only on pairs 0-4,
            # which normalized long ago — runs during pair-5's chain.
            proj_ctr[0] += 1
            if proj_ctr[0] % 5 < 3:
                psw = ps_s.tile([128, 1024], F32, tag="s", name="pso")
            else:
                psw = ps_w.tile([128, 512], F32, tag="w", name="psow")
            pso = psw[:, 0:512]
            for ct in range(CT - 1):
                nc.tensor.matmul(
                    pso[:, 0:hi - lo],
                    lhsT=attnU[ct][:, nt * 128:(nt + 1) * 128],
                    rhs=pwTs[:, ct, lo:hi],
                    start=(ct == 0), stop=False)
            return psw

        def emit_proj_finish(psw, osb, i2, nt, lo, hi):
            pso = psw[:, 0:512]
            nc.tensor.matmul(
                pso[:, 0:hi - lo],
                lhsT=attnU[CT - 1][:, nt * 128:(nt + 1) * 128],
                rhs=pwTs[:, CT - 1, lo:hi],
                start=False, stop=True, skip_group_check=True)
            nc.vector.tensor_add(
                out=osb[:, i2, lo:hi], in0=pso[:, 0:hi - lo],
                in1=pbb[:, lo:hi])

        def emit_out_dma(g, osb):
            eng = nc.sync if g % 2 == 0 else nc.scalar
            eng.dma_start(
                out=out[g * 256:(g + 1) * 256, :].rearrange(
                    "(t p) c -> p t c", p=128),
                in_=osb)

        units = [(g, i2, lohi) for g in range(4) for i2 in range(2)
                 for lohi in ((0, 512), (512, 768))]
        pending = []
        osb_of = {}
        done_of = {g: 0 for g in range(4)}

        def finish_one():
            psw, g, osb, i2, nt, lo, hi = pending.pop(0)
            emit_proj_finish(psw, osb, i2, nt, lo, hi)
            done_of[g] += 1
            if done_of[g] == 4:
                emit_out_dma(g, osb)

        for (g, i2, (lo, hi)) in units:
            if g not in osb_of:
                osb_of[g] = osbp.tile([128, 2, C], F32, tag="osb", name="osb")
            if len(pending) >= 5:
                finish_one()
            nt = g * 2 + i2
            pending.append(
                (emit_proj_partial(nt, lo, hi), g, osb_of[g], i2, nt, lo, hi))
        while pending:
            finish_one()

    _split_dma_waits(nc)
    return nc


_NC_CACHE = None


def _get_nc():
    global _NC_CACHE
    if _NC_CACHE is None:
        _NC_CACHE = build_kernel(
            bass.Bass("TRN2", target_bir_lowering=False, debug=False))
    return _NC_CACHE


def make_in_maps(inputs: dict) -> list[dict]:
    """Host-side shard/layout prep: transpose per chosen layout, fold biases."""
    x = np.asarray(inputs["x"], dtype=np.float32)
    qkv_w = np.asarray(inputs["qkv_w"], dtype=np.float32)
    qkv_b = np.asarray(inputs["qkv_b"], dtype=np.float32)
    proj_w = np.asarray(inputs["proj_w"], dtype=np.float32)
    proj_b = np.asarray(inputs["proj_b"], dtype=np.float32)
    shared = {
        "wT": np.ascontiguousarray(qkv_w.T),
        "pwT": np.ascontiguousarray(proj_w.T),
        "qb": np.ascontiguousarray(qkv_b[0:C].reshape(CT, 128).T),
        # V bias folded through proj (attention rows sum to 1)
        "pb": np.ascontiguousarray(proj_b + proj_w @ qkv_b[2 * C:D3]),
    }
    return [{"xT": np.ascontiguousarray(x[b].T), **shared} for b in range(B)]


def kernel(**inputs: np.ndarray) -> np.ndarray:
    nc = _get_nc()
    in_maps = make_in_maps(inputs)
    res = run_bass_kernel_spmd(nc, in_maps, core_ids=list(range(B)))
    return np.stack([r["out"] for r in res.results]).astype(np.float32)


if __name__ == "__main__":
    from reference import setup_inputs, reference

    inputs = {k: np.asarray(v) for k, v in setup_inputs().items()}
    got = kernel(**inputs)
    exp = np.asarray(reference(**inputs))
    err = np.abs(got - exp)
    print("abs err max:", err.max(), "ref absmax:", np.abs(exp).max())
    print("rel(absmax):", err.max() / np.abs(exp).max())


# revision 36
# speedup vs baseline: 1.0488x; 1.0034x over previous
"""Multi-head attention Trainium2 Bass kernel (v2 — pipelined).

Problem: x[8,1024,768], qkv_w[2304,768], qkv_b[2304], proj_w[768,768],
proj_b[768] -> out[8,1024,768]  (12 heads, head_dim 64, softmax scale 1/8).

Sharding: data-parallel over batch — one batch element per NeuronCore.
Host-side layout prep (part of the sharding strategy): x is passed
transposed per core (xT[c,n]), weights transposed (wT[c,d'], pwT[c,c']).
Two mathematically-exact simplifications:
  - K bias dropped (softmax is invariant to a per-query constant shift).
  - V bias folded into the proj bias: pb = proj_b + proj_w @ v_bias
    (attention rows sum to 1, so the V bias becomes a constant output add).

Per-core pipeline (matmuls bf16, fp32 PSUM; the attention phase is
paced by the ACT-engine exp stream, everything else hides behind it):
  1. Weights cast-DMA'd f32->bf16 via SWDGE (gpsimd); x split across the
     sync HWDGE queue (f32 + DVE cast) and SWDGE so both run in parallel.
  2. Q/K produced transposed qT/kT[d,n]; V natural [n,d] with a ones
     column per head (yields the softmax denominator for free in PV).
  3. Per head-pair: scores sT[j,i] = kT.T@qT into a 3-deep ring of
     [128,1024] PSUM tiles (decouples PE from the exp drain); exp on ACT;
     the 4 PV passes (half x ic) of pair t are deferred into pair t+1's
     jt loop as PE fillers, interleaved with Q/K for pair t+2.  All
     staged tensors are split into per-stage tiles because Tile's RAW
     tracking is tile-coarse.
  4. Denominators: DVE reciprocal costs ~6.4ns per free-dim element, so
     the den rows are resprayed to [128, 16] by DMA, recip'd, then
     DMA-broadcast (via DRAM) across each 64-partition half.
  5. proj reads normalized attnU with a split accumulation (ct 0-4
     partials, ct 5 finisher) so it can start while the last pair's
     normalize chain drains; output over 4 DMAs on sync/scalar queues.
"""

import sys

if "/opt/trn_rl_repo" not in sys.path:
    sys.path.insert(0, "/opt/trn_rl_repo")

from contextlib import ExitStack

import numpy as np

import concourse.bass as bass
import concourse.tile as tile
from concourse import mybir
from concourse.bass_utils import run_bass_kernel_spmd

F32 = mybir.dt.float32
BF16 = mybir.dt.bfloat16
AF = mybir.ActivationFunctionType


def _split_dma_waits(nc: bass.Bass):
    """TRN2 instruction encodings hold at most 1 sync-wait (EventSemaphore: 2),
    but Tile can attach several. Hoist all but one wait onto single-wait NoOps
    inserted just before on the same engine."""
    for f in nc.m.functions:
        for blk in f.blocks:
            insts = blk.instructions
            i = 0
            while i < len(insts):
                inst = insts[i]
                limit = 2 if isinstance(inst, mybir.InstEventSemaphore) else 1
                if (inst.sync_info is not None
                        and len(inst.sync_info.on_wait) > limit):
                    waits = list(inst.sync_info.on_wait)
                    inst.sync_info = mybir.SyncInfo(
                        on_wait=waits[-limit:],
                        on_update=list(inst.sync_info.on_update))
                    for w in waits[:-limit]:
                        nop = mybir.InstNoOp(
                            name=nc.get_next_instruction_name(),
                            ins=[], outs=[])
                        nop.engine = inst.engine
                        nop.sync_info = mybir.SyncInfo(
                            on_wait=[w], on_update=[])
                        insts.insert(i, nop)
                        i += 1
                i += 1


B, N, C = 8, 1024, 768
H, HD = 12, 64
D3 = 3 * C
SCALE = HD ** -0.5
NT = N // 128   # 8 token tiles
CT = C // 128   # 6 channel tiles
NPAIR = H // 2  # 6 head pairs


def build_kernel(nc: bass.Bass):
    xT = nc.dram_tensor("xT", [C, N], F32, kind="ExternalInput").ap()
    wT = nc.dram_tensor("wT", [C, D3], F32, kind="ExternalInput").ap()
    pwT = nc.dram_tensor("pwT", [C, C], F32, kind="ExternalInput").ap()
    qb = nc.dram_tensor("qb", [128, CT], F32, kind="ExternalInput").ap()
    pb = nc.dram_tensor("pb", [C], F32, kind="ExternalInput").ap()
    out = nc.dram_tensor("out", [N, C], F32, kind="ExternalOutput").ap()

    def bcast_ap(src: bass.AP, parts: int) -> bass.AP:
        return bass.AP(tensor=src.tensor, offset=src.offset,
                       ap=[[0, parts], *src.ap])

    with tile.TileContext(nc) as tc, ExitStack() as ctx:
        consts = ctx.enter_context(tc.tile_pool(name="consts", bufs=1))
        expp0 = ctx.enter_context(tc.tile_pool(name="expp0", bufs=10))
        expp1 = ctx.enter_context(tc.tile_pool(name="expp1", bufs=14))
        dstgp = ctx.enter_context(tc.tile_pool(name="dstg", bufs=2))
        rbcp = ctx.enter_context(tc.tile_pool(name="rbc", bufs=1))
        osbp = ctx.enter_context(tc.tile_pool(name="osb", bufs=2))
        ps_s = ctx.enter_context(tc.tile_pool(name="ps_s", bufs=3, space="PSUM"))
        ps_w = ctx.enter_context(tc.tile_pool(name="ps_w", bufs=2, space="PSUM"))
        dram = ctx.enter_context(tc.tile_pool(name="dram", bufs=2, space="DRAM"))

        # ---- persistent bf16 operands --------------------------------
        # Tile RAW tracking is tile-coarse (a read waits on ALL prior
        # writes to the tile), so anything written in stages is split
        # into per-stage tiles to avoid false serialization.
        xTs = consts.tile([128, CT, N], BF16)       # x.T  [c, n]
        wq0 = consts.tile([128, CT, 128], BF16)     # q cols, d-tile 0
        wk0 = consts.tile([128, CT, 128], BF16)     # k cols, d-tile 0
        wqr = consts.tile([128, CT, 5 * 128], BF16)  # q cols, d-tiles 1-5
        wkr = consts.tile([128, CT, 5 * 128], BF16)  # k cols, d-tiles 1-5
        wv = consts.tile([128, CT, C], BF16)        # qkv_w.T v cols
        pwTs = consts.tile([128, CT, C], BF16)      # proj_w.T
        qTt = [consts.tile([128, N], BF16, name=f"qT{i}") for i in range(CT)]
        kTt = [consts.tile([128, N], BF16, name=f"kT{i}") for i in range(CT)]
        v_sb = consts.tile([128, NT, H, HD + 1], BF16)  # v + ones col
        attnU = [consts.tile([128, N], BF16, name=f"aU{i}") for i in range(CT)]
        qbs = consts.tile([128, CT], F32)           # q bias [p, t]
        pbb = consts.tile([128, C], F32)            # proj(+v) bias bcast
        wrm = consts.tile([1, 8], F32)
        wrm2 = consts.tile([1, 8], BF16, name="wrm2")

        # ---- warmup exp: pull the ACT table load to t=0 ---------------
        nc.vector.memset(wrm, 0.0)
        nc.scalar.activation(out=wrm2, in_=wrm, func=AF.Exp, scale=SCALE)
        nc.vector.memset(v_sb[:, :, :, HD:HD + 1], 1.0)

        # ---- input DMAs ----------------------------------------------
        # x + weights via SWDGE cast-DMA (f32 -> bf16), ordered so pair 0
        # can start asap: x per-ct, then just the d-tile-0 q/k columns,
        # then wv (pair-0 V fillers), then the remaining q/k, then proj_w.
        # x: low half on sync HWDGE (f32 + DVE cast), high half on SWDGE —
        # the two queues run in parallel so x is resident ~5us sooner.
        nc.gpsimd.dma_start(
            out=xTs[:, 2:6, :],
            in_=xT[256:768, :].rearrange("(ct p) n -> p ct n", p=128))
        xs = consts.tile([128, 2, N], F32, name="xstage")
        nc.sync.dma_start(
            out=xs, in_=xT[0:256, :].rearrange("(ct p) n -> p ct n", p=128))
        nc.vector.tensor_copy(out=xTs[:, 0:2, :], in_=xs)
        # d-tile-0 q/k columns on the scalar HWDGE queue (f32 + DVE cast);
        # the slow-descriptor bias loads go LAST on that queue.
        wq0s = consts.tile([128, CT, 128], F32, name="wq0s")
        wk0s = consts.tile([128, CT, 128], F32, name="wk0s")
        nc.scalar.dma_start(
            out=wq0s, in_=wT[:, 0:128].rearrange("(ct p) d -> p ct d", p=128))
        nc.vector.tensor_copy(out=wq0, in_=wq0s)
        nc.scalar.dma_start(
            out=wk0s,
            in_=wT[:, C:C + 128].rearrange("(ct p) d -> p ct d", p=128))
        nc.vector.tensor_copy(out=wk0, in_=wk0s)
        nc.scalar.dma_start(out=qbs, in_=qb)
        nc.scalar.dma_start(out=pbb, in_=bcast_ap(pb, 128))
        nc.gpsimd.dma_start(
            out=wv, in_=wT[:, 2 * C:D3].rearrange("(ct p) d -> p ct d", p=128))
        nc.gpsimd.dma_start(
            out=wqr,
            in_=wT[:, 128:C].rearrange("(ct p) d -> p ct d", p=128))
        nc.gpsimd.dma_start(
            out=wkr,
            in_=wT[:, C + 128:2 * C].rearrange("(ct p) d -> p ct d", p=128))
        nc.gpsimd.dma_start(
            out=pwTs, in_=pwT.rearrange("(ct p) d -> p ct d", p=128))

        # ---- emit helpers --------------------------------------------
        def emit_q(t, ic):
            ps = ps_w.tile([128, 512], F32, tag="w", name="psq")
            w = wq0 if t == 0 else wqr
            toff = 0 if t == 0 else (t - 1) * 128
            for ct in range(CT):
                nc.tensor.matmul(
                    ps, lhsT=w[:, ct, toff:toff + 128],
                    rhs=xTs[:, ct, ic * 512:(ic + 1) * 512],
                    start=(ct == 0), stop=(ct == CT - 1))
            nc.vector.tensor_scalar_add(
                out=qTt[t][:, ic * 512:(ic + 1) * 512], in0=ps,
                scalar1=qbs[:, t:t + 1])

        def emit_k(t, ic):
            ps = ps_w.tile([128, 512], F32, tag="w", name="psk")
            w = wk0 if t == 0 else wkr
            toff = 0 if t == 0 else (t - 1) * 128
            for ct in range(CT):
                nc.tensor.matmul(
                    ps, lhsT=w[:, ct, toff:toff + 128],
                    rhs=xTs[:, ct, ic * 512:(ic + 1) * 512],
                    start=(ct == 0), stop=(ct == CT - 1))
            nc.vector.tensor_copy(
                out=kTt[t][:, ic * 512:(ic + 1) * 512], in_=ps)

        def emit_v_chunk(t, half):
            lo, hi, h0, hn = ((0, 512, 0, 8), (512, 768, 8, 4))[half]
            ps = ps_w.tile([128, 512], F32, tag="w", name="psv")
            for ct in range(CT):
                nc.tensor.matmul(
                    ps[:, 0:hi - lo],
                    lhsT=xTs[:, ct, t * 128:(t + 1) * 128],
                    rhs=wv[:, ct, lo:hi],
                    start=(ct == 0), stop=(ct == CT - 1))
            nc.vector.tensor_copy(
                out=v_sb[:, t, h0:h0 + hn, 0:HD],
                in_=ps[:, 0:hi - lo].rearrange("p (h d) -> p h d", h=hn))

        def emit_scores(t, jt):
            # per-half scores tiles (double-buffered in PSUM) so the exp on
            # ACT overlaps the next scores matmuls instead of serializing.
            s_pair = [ps_s.tile([128, 1024], F32, tag="s", name="s")
                      for _ in range(2)]
            for ic in range(2):
                for h in range(2):
                    nc.tensor.matmul(
                        s_pair[h][:, ic * 512:(ic + 1) * 512],
                        lhsT=kTt[t][64 * h:64 * h + 64, jt * 128:(jt + 1) * 128],
                        rhs=qTt[t][64 * h:64 * h + 64, ic * 512:(ic + 1) * 512],
                        start=True, stop=True)
            e_pair = []
            for h in range(2):
                # separate rings per half: h0 tiles are freed (by the
                # deferred PV passes) two jt-steps earlier than h1 tiles,
                # and a shared FIFO ring would serialize on the late ones.
                e = (expp0 if h == 0 else expp1).tile(
                    [128, 1024], BF16, tag=f"e{h}", name="e")
                nc.scalar.activation(out=e, in_=s_pair[h], func=AF.Exp,
                                     scale=SCALE)
                e_pair.append(e)
            return e_pair

        def emit_pv_pass(t, h, ic, e_list, dstg_t):
            ow = ps_w.tile([128, 512], F32, tag="w", name="o")
            o = ow[0:65, :]
            for jt in range(NT):
                nc.tensor.matmul(
                    o, lhsT=v_sb[:, jt, 2 * t + h, :],
                    rhs=e_list[jt][h][:, ic * 512:(ic + 1) * 512],
                    start=(jt == 0), stop=(jt == NT - 1))
            # den copy first: it gates the (serial-DMA) normalize chain,
            # while the attnU copy is only needed by the later proj.
            nc.vector.tensor_copy(
                out=dstg_t[64:65, h, ic * 512:(ic + 1) * 512],
                in_=o[64:65, :])
            nc.vector.tensor_copy(
                out=attnU[t][64 * h:64 * h + 64, ic * 512:(ic + 1) * 512],
                in_=o[0:64, :])

        def emit_normalize_half(t, h, dstg_t, eng):
            # One head's den row lives at partition 64 of dstg.  DVE
            # reciprocal costs ~6.4ns per FREE-dim element, so scatter the
            # 1024 denominators to [128, 8] (SBUF->SBUF DMA respray) for
            # the reciprocal, then DMA-broadcast the RECIPROCALS across
            # this head's 64-partition half.
            rsm = rbcp.tile([128, 8], F32, tag=f"rs{h}", name="rsm")
            eng.dma_start(out=rsm, in_=dstg_t[64:65, h, :])
            nc.vector.reciprocal(out=rsm, in_=rsm)
            dsc = dram.tile([N], F32, tag=f"dsc{h}", name="dsc")
            eng.dma_start(out=dsc.rearrange("(p f) -> p f", p=128), in_=rsm)
            rbc = rbcp.tile([128, N], F32, tag=f"r{h}", name="rbc")
            eng.dma_start(out=rbc[64 * h:64 * h + 64, :], in_=bcast_ap(dsc, 64))
            nc.vector.tensor_mul(
                out=attnU[t][64 * h:64 * h + 64, :],
                in0=attnU[t][64 * h:64 * h + 64, :],
                in1=rbc[64 * h:64 * h + 64, :])

        # ---- phase A: first Q/K ---------------------------------------
        emit_q(0, 0)
        emit_q(0, 1)
        emit_k(0, 0)  # K ic1 (keys 512:1024) is deferred to pair-0 jt0

        # ---- attention: pipelined pairs ------------------------------
        # Filler schedule per jt (pairs >= 1), chosen to spread PE work
        # evenly so HAM never re-throttles:
        #   jt0,jt1: prev pair's h0 PV passes     jt2,jt3: Q(t+1)
        #   jt4,jt5: prev pair's h1 PV passes     jt6,jt7: K(t+1)
        #   jt2: normalize prev h0   jt6: normalize prev h1
        e_hist: dict[int, list] = {}
        dstg_hist: dict[int, object] = {}
        for t in range(NPAIR):
            dstg_hist[t] = dstgp.tile([65, 2, N], F32, tag="d", name="dstg")
            e_list = []
            for jt in range(NT):
                if t == 0:
                    # scores first: the V fillers gate on the (late) wv DMA
                    # and must not block them in PE program order.
                    e_list.append(emit_scores(t, jt))
                    if jt == 0:
                        emit_k(0, 1)
                    if jt >= 2:
                        for vq in range(3):
                            vi = (jt - 2) * 3 + vq
                            if vi < 16:
                                emit_v_chunk(vi // 2, vi % 2)
                    if jt >= 4:
                        (emit_q if jt < 6 else emit_k)(1, jt % 2)
                else:
                    # fillers first: PE chews these while ACT drains the
                    # previous jt's scores tiles.
                    if jt in (0, 1, 4, 5):
                        h, ic = divmod(jt, 4)
                        emit_pv_pass(t - 1, h, ic, e_hist[t - 1],
                                     dstg_hist[t - 1])
                    elif t + 1 < NPAIR:
                        (emit_q if jt < 4 else emit_k)(t + 1, jt % 2)
                    if jt == 2:
                        emit_normalize_half(t - 1, 0, dstg_hist[t - 1],
                                            nc.sync)
                    elif jt == 6:
                        emit_normalize_half(t - 1, 1, dstg_hist[t - 1],
                                            nc.sync)
                    e_list.append(emit_scores(t, jt))
            e_hist[t] = e_list
            if t - 2 >= 0:
                del e_hist[t - 2]  # release python refs (slots recycle anyway)

        # ---- tail: last pair's PV + normalize ------------------------
        tl = NPAIR - 1
        for h in (1, 0):
            for ic in range(2):
                emit_pv_pass(tl, h, ic, e_hist[tl], dstg_hist[tl])
        # single combined chain for the last pair: one scatter+recip+store,
        # two broadcasts on parallel queues, one full-width mul — the last
        # mul gates proj's ct5 matmuls, so chain length is the tail.
        dstg_t = dstg_hist[tl]
        rsm2 = rbcp.tile([128, 16], F32, tag="rs2", name="rsm2")
        nc.sync.dma_start(
            out=rsm2, in_=dstg_t[64:65, :, :].rearrange("p h n -> p (h n)"))
        nc.vector.reciprocal(out=rsm2, in_=rsm2)
        dsc5 = dram.tile([2, N], F32, tag="dsc5", name="dsc5")
        nc.sync.dma_start(
            out=dsc5.rearrange("h n -> (h n)").rearrange("(p f) -> p f",
                                                         p=128),
            in_=rsm2)
        rbc5 = rbcp.tile([128, N], F32, tag="r5", name="rbc5")
        nc.sync.dma_start(out=rbc5[0:64, :], in_=bcast_ap(dsc5[0, :], 64))
        nc.scalar.dma_start(out=rbc5[64:128, :], in_=bcast_ap(dsc5[1, :], 64))
        nc.vector.tensor_mul(
            out=attnU[tl][:, :], in0=attnU[tl][:, :], in1=rbc5)

        # ---- output projection ---------------------------------------
        # ct order puts the straggler pair (ct=5) last, so each chunk's
        # first 5 matmuls can run while its normalize chain drains.
        proj_ctr = [0]

        def emit_proj_partial(nt, lo, hi):
            # ct 0..4 partial accumulation: depends only on pairs 0-4,
            # which normalized long ago — runs during pair-5's chain.
            proj_ctr[0] += 1
            if proj_ctr[0] % 5 < 3:
                psw = ps_s.tile([128, 1024], F32, tag="s", name="pso")
            else:
                psw = ps_w.tile([128, 512], F32, tag="w", name="psow")
            pso = psw[:, 0:512]
            for ct in range(CT - 1):
                nc.tensor.matmul(
                    pso[:, 0:hi - lo],
                    lhsT=attnU[ct][:, nt * 128:(nt + 1) * 128],
                    rhs=pwTs[:, ct, lo:hi],
                    start=(ct == 0), stop=False)
            return psw

        def emit_proj_finish(psw, osb, i2, nt, lo, hi):
            pso = psw[:, 0:512]
            nc.tensor.matmul(
                pso[:, 0:hi - lo],
                lhsT=attnU[CT - 1][:, nt * 128:(nt + 1) * 128],
                rhs=pwTs[:, CT - 1, lo:hi],
                start=False, stop=True, skip_group_check=True)
            nc.vector.tensor_add(
                out=osb[:, i2, lo:hi], in0=pso[:, 0:hi - lo],
                in1=pbb[:, lo:hi])

        def emit_out_dma(g, osb):
            eng = nc.sync if g % 2 == 0 else nc.scalar
            eng.dma_start(
                out=out[g * 256:(g + 1) * 256, :].rearrange(
                    "(t p) c -> p t c", p=128),
                in_=osb)

        units = [(g, i2, lohi) for g in range(4) for i2 in range(2)
                 for lohi in ((0, 512), (512, 768))]
        pending = []
        osb_of = {}
        done_of = {g: 0 for g in range(4)}

        def finish_one():
            psw, g, osb, i2, nt, lo, hi = pending.pop(0)
            emit_proj_finish(psw, osb, i2, nt, lo, hi)
            done_of[g] += 1
            if done_of[g] == 4:
                emit_out_dma(g, osb)

        for (g, i2, (lo, hi)) in units:
            if g not in osb_of:
                osb_of[g] = osbp.tile([128, 2, C], F32, tag="osb", name="osb")
            if len(pending) >= 5:
                finish_one()
            nt = g * 2 + i2
            pending.append(
                (emit_proj_partial(nt, lo, hi), g, osb_of[g], i2, nt, lo, hi))
        while pending:
            finish_one()

    _split_dma_waits(nc)
    return nc


_NC_CACHE = None


def _get_nc():
    global _NC_CACHE
    if _NC_CACHE is None:
        _NC_CACHE = build_kernel(
            bass.Bass("TRN2", target_bir_lowering=False, debug=False))
    return _NC_CACHE


def make_in_maps(inputs: dict) -> list[dict]:
    """Host-side shard/layout prep: transpose per chosen layout, fold biases."""
    x = np.asarray(inputs["x"], dtype=np.float32)
    qkv_w = np.asarray(inputs["qkv_w"], dtype=np.float32)
    qkv_b = np.asarray(inputs["qkv_b"], dtype=np.float32)
    proj_w = np.asarray(inputs["proj_w"], dtype=np.float32)
    proj_b = np.asarray(inputs["proj_b"], dtype=np.float32)
    shared = {
        "wT": np.ascontiguousarray(qkv_w.T),
        "pwT": np.ascontiguousarray(proj_w.T),
        "qb": np.ascontiguousarray(qkv_b[0:C].reshape(CT, 128).T),
        # V bias folded through proj (attention rows sum to 1)
        "pb": np.ascontiguousarray(proj_b + proj_w @ qkv_b[2 * C:D3]),
    }
    return [{"xT": np.ascontiguousarray(x[b].T), **shared} for b in range(B)]


def kernel(**inputs: np.ndarray) -> np.ndarray:
    nc = _get_nc()
    in_maps = make_in_maps(inputs)
    res = run_bass_kernel_spmd(nc, in_maps, core_ids=list(range(B)))
    return np.stack([r["out"] for r in res.results]).astype(np.float32)


if __name__ == "__main__":
    from reference import setup_inputs, reference

    inputs = {k: np.asarray(v) for k, v in setup_inputs().items()}
    got = kernel(**inputs)
    exp = np.asarray(reference(**inputs))
    err = np.abs(got - exp)
    print("abs err max:", err.max(), "ref absmax:", np.abs(exp).max())
    print("rel(absmax):", err.max() / np.abs(exp).max())
